# revision 6
# baseline (speedup 1.0000x reference)
"""Trainium2 Bass kernel: dual-softmax ("contrast") multi-head self-attention.

Problem (per full input):
  x, y: (4, 1024, 1024) f32; Wq/Wk/Wv: (1024, 1024) f32, nh=16 heads, dk=dv=64.
  q = x @ Wq.T, k = x @ Wk.T, v = y @ Wv.T  (split heads)
  dist   = softmax(q k^T / 8)
  c_att  = softmax(1 - dist) @ v      (== softmax(-dist) @ v, shift invariance)
  att    = softmax(dist) @ v
  returns (c_att, att), each (4, 1024, 1024) f32.

Sharding: 8 cores = 4 batches x 2 head-groups (8 heads each). Each core gets
x[b], y[b] and a 512-row slice of each weight; returns (c_att, att) slices
[1024, 512].

V2 design (bf16 matmuls, transpose-free scores):
  Loads arrive pre-transposed: DMA f32 -> DVE cast bf16 -> SBUF->SBUF
  dma_start_transpose (xbar) -> xt/yt/wqt/wkt/wvt in [contract-dim, free] form.
  QT = wqt.T @ xt, KT = wkt.T @ xt   [feat, tok] bf16; V via lhsT=yt slices.
  Per head, k-major throughout (no PE transposes of the score matrix):
    S^T[k,q] = KT_h^T QT_h via matmul (16 x 512-col bf16 MMs)
    E1T = exp(S^T/8)                  [ACT, PSUM->SBUF bf16]
    rowsum1 (over k = partitions) via ones[128,128]-stationary matmul ->
      replicated [128,1024] PSUM; r1 = recip_approx_fast (f32) -> bf16
    D = E1T * r1 (broadcast along kb)  [DVE TT bf16, in place]
    E3T = exp(D) [ACT]; E2T = 1/E3T = exp(-D) [DVE recip_approx bf16]
    O3^T/O2^T accumulate via V_aug-stationary (65 rows incl ones col)
    evac bf16 -> dma_start_transpose -> [tok, 65]; divide by col 64 [DVE].
"""

import sys

if "/opt/trn_rl_repo" not in sys.path:
    sys.path.insert(0, "/opt/trn_rl_repo")

from contextlib import ExitStack

import numpy as np

import concourse.bass as bass
from concourse import bacc, mybir
from concourse.bass_utils import run_bass_kernel_spmd
from concourse.dve_ops import RECIP_APPROX_FAST_CONSTS, RECIPROCAL_APPROX_FAST
from concourse.tile import TileContext

F32 = mybir.dt.float32
BF = mybir.dt.bfloat16
EXP = mybir.ActivationFunctionType.Exp
ADD = mybir.AluOpType.add
MUL = mybir.AluOpType.mult

P = 128          # partitions
N = 1024         # tokens
D = 1024         # model dim
NF = 512         # features per core (8 heads x 64)
FH = 8           # heads per core
DK = 64          # head dim
NPT = N // P     # 8 token ptiles
KBN = D // P     # 8 contraction blocks
MB = NF // P     # 4 feature ptiles
HB = KBN // 2    # kb half-batch for elementwise ops

# how many of the 2 per-head [128,4,1024] E2 batches go to ACT exp(-D)
# instead of DVE reciprocal-of-E3 (tune for ACT/DVE balance)
E2_ACT_BATCHES = 0


def build_nc():
    nc = bacc.Bacc("TRN2")
    x_d = nc.dram_tensor("x", [N, D], F32, kind="ExternalInput")
    y_d = nc.dram_tensor("y", [N, D], F32, kind="ExternalInput")
    wq_d = nc.dram_tensor("wq", [NF, D], F32, kind="ExternalInput")
    wk_d = nc.dram_tensor("wk", [NF, D], F32, kind="ExternalInput")
    wv_d = nc.dram_tensor("wv", [NF, D], F32, kind="ExternalInput")
    catt_d = nc.dram_tensor("catt", [N, NF], F32, kind="ExternalOutput")
    att_d = nc.dram_tensor("att", [N, NF], F32, kind="ExternalOutput")

    with TileContext(nc) as tc, ExitStack() as ctx:
        persist = ctx.enter_context(tc.tile_pool(name="persist", bufs=1))
        qt = persist.tile([P, MB, N], BF)        # Q^T: [feat%128, featblk, tok]
        kt = persist.tile([P, MB, N], BF)
        vv = persist.tile([P, NPT, FH, DK + 1], BF)   # V_aug per head
        att_sb = persist.tile([P, NPT, NF], F32)
        catt_sb = persist.tile([P, NPT, NF], F32)
        ones_bf = persist.tile([P, P], BF)
        nc.vector.memset(ones_bf[:], 1.0)
        nc.vector.memset(vv[:, :, :, DK:DK + 1], 1.0)

        # ---------------- setup: transposed loads + projections ----------------
        with ExitStack() as sctx:
            sbp = sctx.enter_context(tc.tile_pool(name="setup", bufs=1))
            raw = sctx.enter_context(tc.tile_pool(name="raw", bufs=3))
            pst = sctx.enter_context(tc.tile_pool(name="pst", bufs=2, space="PSUM"))

            xt = sbp.tile([P, KBN, N], BF, tag="xt")
            yt = sbp.tile([P, KBN, N], BF, tag="yt")
            wqt = sbp.tile([P, KBN, NF], BF, tag="wqt")
            wkt = sbp.tile([P, KBN, NF], BF, tag="wkt")
            wvt = sbp.tile([P, KBN, NF], BF, tag="wvt")

            # loads on the sync queue, transposes on the scalar queue: two
            # independent DMA FIFOs so loads never queue behind transposes
            for src_d, nslab, dst in ((x_d, NPT, xt), (wq_d, MB, wqt),
                                      (wk_d, MB, wkt), (y_d, NPT, yt),
                                      (wv_d, MB, wvt)):
                for s in range(nslab):
                    rw = raw.tile([P, D], F32, tag="raw")
                    nc.sync.dma_start(out=rw[:], in_=src_d[s * P:(s + 1) * P, :])
                    bfs = raw.tile([P, D], BF, tag="bf")
                    nc.vector.tensor_copy(bfs[:], rw[:])
                    nc.scalar.dma_start_transpose(
                        out=dst[:, :, s * P:(s + 1) * P], in_=bfs[:])

            # projections (contraction over model dim d in kb blocks)
            for wi, (wt, out_sb) in enumerate(((wqt, qt), (wkt, kt))):
                for m in range(MB):
                    for ch in range(2):
                        sl = slice(ch * 512, (ch + 1) * 512)
                        ps = pst.tile([P, 512], F32, tag="proj")
                        for kb in range(KBN):
                            nc.tensor.matmul(
                                ps[:],
                                lhsT=wt[:, kb, m * P:(m + 1) * P],
                                rhs=xt[:, kb, sl],
                                start=(kb == 0),
                                stop=(kb == KBN - 1),
                            )
                        if wi == 0:
                            nc.scalar.copy(out_sb[:, m, sl], ps[:])
                        else:
                            nc.vector.tensor_copy(out_sb[:, m, sl], ps[:])
            for i in range(NPT):
                ps = pst.tile([P, 512], F32, tag="proj")
                for kb in range(KBN):
                    nc.tensor.matmul(
                        ps[:],
                        lhsT=yt[:, kb, i * P:(i + 1) * P],
                        rhs=wvt[:, kb, :],
                        start=(kb == 0),
                        stop=(kb == KBN - 1),
                    )
                nc.scalar.copy(
                    vv[:, i, :, 0:DK],
                    ps[:].rearrange("p (h d) -> p h d", h=FH),
                )

        # ---------------- per-head attention ----------------
        e1p = ctx.enter_context(tc.tile_pool(name="e1p", bufs=2))
        rsp = ctx.enter_context(tc.tile_pool(name="rsp", bufs=2))
        e3p = ctx.enter_context(tc.tile_pool(name="e3p", bufs=6))
        e2p = ctx.enter_context(tc.tile_pool(name="e2p", bufs=6))
        osbp = ctx.enter_context(tc.tile_pool(name="osbp", bufs=4))
        otsp = ctx.enter_context(tc.tile_pool(name="otsp", bufs=4))
        smp = ctx.enter_context(tc.tile_pool(name="smp", bufs=24))
        psb = ctx.enter_context(tc.tile_pool(name="psb", bufs=2, space="PSUM"))
        rsb = ctx.enter_context(tc.tile_pool(name="rsb", bufs=1, space="PSUM"))
        pso = ctx.enter_context(tc.tile_pool(name="pso", bufs=2, space="PSUM"))

        cc = RECIP_APPROX_FAST_CONSTS

        for h in range(FH):
            hb, ho = h // 2, (h % 2) * DK
            e1t = e1p.tile([P, KBN, N], BF, tag="e1")
            rs_ps = rsb.tile([P, N], F32, tag="rs")
            # all S^T matmuls first (PE never queues behind an ACT-dependent
            # rowsum matmul), then the rowsum accumulation group
            for kb in range(KBN):
                for ch in range(2):
                    sl = slice(ch * 512, (ch + 1) * 512)
                    s_ps = psb.tile([P, 512], F32, tag="st")
                    nc.tensor.matmul(
                        s_ps[:],
                        lhsT=kt[ho:ho + DK, hb, kb * P:(kb + 1) * P],
                        rhs=qt[ho:ho + DK, hb, sl],
                        start=True,
                        stop=True,
                    )
                    nc.scalar.activation(e1t[:, kb, sl], s_ps[:], EXP,
                                         scale=0.125)
            for kb in range(KBN):
                for ch in range(2):
                    sl = slice(ch * 512, (ch + 1) * 512)
                    nc.tensor.matmul(
                        rs_ps[:, sl],
                        lhsT=ones_bf[:],
                        rhs=e1t[:, kb, sl],
                        start=(kb == 0),
                        stop=(kb == KBN - 1),
                    )
            r1f = rsp.tile([P, N], F32, tag="r1f")
            nc.vector._custom_dve(
                RECIPROCAL_APPROX_FAST, out=r1f[:], in0=rs_ps[:],
                s0=cc["s0"], s1=cc["s1"], imm2=cc["imm2"])
            r1b = rsp.tile([P, N], BF, tag="r1b")
            nc.vector.tensor_copy(r1b[:], r1f[:])
            r1x = r1b.rearrange("p (o n) -> p o n", o=1).broadcast_to([P, 2, N])

            o3_ps = pso.tile([DK + 1, N], F32, tag="o")
            o2_ps = pso.tile([DK + 1, N], F32, tag="o")
            for half in range(4):
                d2 = e1t[:, half * 2:(half + 1) * 2, :]
                # D = dist (in place over E1T)
                nc.vector.tensor_mul(d2, d2, r1x)
                e3 = e3p.tile([P, 2, N], BF, tag="e3")
                nc.scalar.activation(e3[:], d2, EXP)
                e2 = e2p.tile([P, 2, N], BF, tag="e2")
                if half < E2_ACT_BATCHES:
                    nc.scalar.activation(e2[:], d2, EXP, scale=-1.0)
                else:
                    nc.vector._custom_dve(
                        RECIPROCAL_APPROX_FAST, out=e2[:], in0=e3[:],
                        s0=cc["s0"], s1=cc["s1"], imm2=cc["imm2"])
                for j in range(2):
                    kb = half * 2 + j
                    for ch in range(2):
                        sl = slice(ch * 512, (ch + 1) * 512)
                        nc.tensor.matmul(
                            o3_ps[:, sl], lhsT=vv[:, kb, h, :],
                            rhs=e3[:, j, sl],
                            start=(kb == 0), stop=(kb == KBN - 1),
                        )
                        nc.tensor.matmul(
                            o2_ps[:, sl], lhsT=vv[:, kb, h, :],
                            rhs=e2[:, j, sl],
                            start=(kb == 0), stop=(kb == KBN - 1),
                        )

            for o_ps, out_t in ((o3_ps, att_sb), (o2_ps, catt_sb)):
                osb_t = osbp.tile([80, N], BF, tag="osb")
                nc.vector.tensor_copy(osb_t[0:DK + 1, :], o_ps[:])
                ot = otsp.tile([P, NPT, 80], BF, tag="ots")
                nc.sync.dma_start_transpose(out=ot[:], in_=osb_t[:])
                # batched normalization: one reciprocal of the 8 rowsums, one
                # broadcast multiply over all 8 token tiles
                rr = smp.tile([P, NPT], F32, tag="rr")
                nc.vector.reciprocal(rr[:], ot[:, :, DK])
                rrx = rr.rearrange("p (i o) -> p i o", o=1).broadcast_to(
                    [P, NPT, DK])
                nc.vector.tensor_mul(
                    out_t[:].rearrange("p i (h8 d) -> p i h8 d", d=DK)
                    [:, :, h, :], ot[:, :, 0:DK], rrx)

        for i in range(NPT):
            nc.sync.dma_start(out=att_d[i * P:(i + 1) * P, :], in_=att_sb[:, i, :])
            nc.sync.dma_start(out=catt_d[i * P:(i + 1) * P, :], in_=catt_sb[:, i, :])

    nc.finalize()
    return nc


_NC_CACHE = {}


def _get_nc():
    if "nc" not in _NC_CACHE:
        _NC_CACHE["nc"] = build_nc()
    return _NC_CACHE["nc"]


def _make_in_maps(x, y, Wq, Wk, Wv):
    x = np.ascontiguousarray(np.asarray(x, dtype=np.float32))
    y = np.ascontiguousarray(np.asarray(y, dtype=np.float32))
    Wq = np.ascontiguousarray(np.asarray(Wq, dtype=np.float32))
    Wk = np.ascontiguousarray(np.asarray(Wk, dtype=np.float32))
    Wv = np.ascontiguousarray(np.asarray(Wv, dtype=np.float32))
    in_maps = []
    for c in range(8):
        b, h0 = c // 2, (c % 2) * 8
        rows = slice(h0 * DK, h0 * DK + NF)
        in_maps.append({
            "x": x[b],
            "y": y[b],
            "wq": np.ascontiguousarray(Wq[rows]),
            "wk": np.ascontiguousarray(Wk[rows]),
            "wv": np.ascontiguousarray(Wv[rows]),
        })
    return in_maps


def run_cores(x, y, Wq, Wk, Wv, trace=False, tmpdir=None):
    nc = _get_nc()
    res = run_bass_kernel_spmd(
        nc, _make_in_maps(x, y, Wq, Wk, Wv), core_ids=list(range(8)),
        trace=trace, tmpdir=tmpdir,
    )
    B = 4
    c_att = np.empty((B, N, 2 * NF), dtype=np.float32)
    att = np.empty((B, N, 2 * NF), dtype=np.float32)
    for c, r in enumerate(res.results):
        b, cols = c // 2, slice((c % 2) * NF, (c % 2) * NF + NF)
        c_att[b][:, cols] = r["catt"]
        att[b][:, cols] = r["att"]
    return (c_att, att), res


def kernel(x, y, Wq, Wk, Wv):
    out, _ = run_cores(x, y, Wq, Wk, Wv)
    return out
